# revision 31
# baseline (speedup 1.0000x reference)
"""Trainium2 Bass kernel for nn_MemoryGame (scatter_memory).

Math (see reference):
    P = 8192, T = 4 timesteps, N_ITER = 50 attractor iterations.
    per t: h0 = f_p(tile(g_t, 128));  50x: h = f_p(kappa*h + h*(h@M))
           p = outer(x_t, g_t).ravel()
           loss_t = sum|p - h|
           M = lamda*M + yita*outer(p+h, p-h)
    output = sum_t loss_t   (scalar, fp32)

Distribution: M column-sharded over 8 cores (core k owns columns
[k*1024,(k+1)*1024)).  Each core computes its 1024-slice of a = h@M and
an AllGather rebuilds the full h each iteration.

Numerics (measured on CPU, exact-arithmetic sim): storing M in fp16,
rounding h to fp16 each iteration, and accumulating the DVE-side
partials in fp16 all land the final loss within ~5e-4 of the fp32
reference (tolerance 2e-2), so the whole 16 MiB fp16 shard stays
SBUF-resident: ZERO per-iteration HBM traffic.

Per iteration the GEMV is split two ways:
  - contraction chunks (64 of [128 rows, 1024 cols]): N_PE of them run on
    the PE as single-pass fp16 matmuls (h16 stationary, M16 moving);
    the rest accumulate on the DVE via fused scalar_tensor_tensor
    (acc16 = mc*h_c + acc16), partition-reduced by a ones-vector matmul
    into the same PSUM bank as the PE's partials.
  - output columns are processed half0 then half1, so half0's pointwise
    + AllGather launch ~mid-iteration and overlap half1's compute.

Layout: contraction index i = g(c)*128 + p (chunk-major, permuted so
chunks 0-31 hold rows with (i mod 1024) < 512 = the columns carried by
the half-0 AllGather).  The gathered [1,4096] buffer then IS the h
values in [32 chunks, 128 p] order: one contiguous DMA + a PE transpose
+ an ACT fp16 copy rebuild h16_sb[:, chunk] with no strided traffic.
Group-0 chunks only need AG0, so the next iteration's group-0 work
starts while AG1 is still in flight (deferred-assembly software
pipeline); the Hebbian update is one fused DVE op per chunk.
"""

import os
import numpy as np

N_CORES = 8
P_DIM = 8192
NXD, NGD = 128, 64
T_STEPS = 4
N_ITER = 50
KAPPA, LAMDA, YITA = 0.8, 0.9, 0.1
NEG = 0.01

NCHUNK = 64                 # contraction chunks (128 rows each)
J_LOC = P_DIM // N_CORES    # 1024 columns per core
ND_G0 = 10                  # DVE chunks in group0 (starts early, off asm0)
ND_G1 = 4                   # DVE chunks in group1 (kept small: group1 work sits
                            # on the post-asm1 critical path)

_cache = {}


def g_perm(c):
    """chunk -> row-block permutation: chunks 0-31 land in column-half0."""
    if c < 32:
        return (c // 4) * 8 + (c % 4)
    c -= 32
    return (c // 4) * 8 + 4 + (c % 4)


# chunk engine assignment: within each group the first chunks go to the
# PE, the last ND2 to the DVE.
PE_G0 = list(range(0, 32 - ND_G0))
DVE_G0 = list(range(32 - ND_G0, 32))
PE_G1 = list(range(32, 64 - ND_G1))
DVE_G1 = list(range(64 - ND_G1, 64))


def _f_p(v):
    c = np.clip(v, -1.0, 1.0)
    return np.where(c >= 0, c, NEG * c).astype(np.float32)


def build_program(debug_h=False, n_iter=None, t_run=None):
    import concourse.bacc as bacc
    import concourse.mybir as mybir
    import concourse.tile as tile

    if n_iter is None:
        n_iter = N_ITER
    if t_run is None:
        t_run = T_STEPS

    f32 = mybir.dt.float32
    f16 = mybir.dt.float16
    ALU = mybir.AluOpType

    nc = bacc.Bacc(None, target_bir_lowering=False, num_devices=N_CORES)

    AF = mybir.ActivationFunctionType
    # register KAPPA so activation(bias=KAPPA) finds a const AP
    kapc = nc.alloc_sbuf_tensor("const-kappa", [128, 1], f32)
    nc.gpsimd.memset(kapc.ap(), float(KAPPA))
    nc.const_aps.aps[(f32, float(KAPPA))] = kapc.ap()
    nc.all_engine_barrier()

    # ---- I/O ----
    m16_in = nc.dram_tensor("m16_in", [128, NCHUNK * J_LOC], f16, kind="ExternalInput")
    h0_sb_in = nc.dram_tensor("h0_sb_in", [T_STEPS, 128, NGD], f32, kind="ExternalInput")
    h0_row_in = nc.dram_tensor("h0_row_in", [T_STEPS, 1, J_LOC], f32, kind="ExternalInput")
    p_sb_in = nc.dram_tensor("p_sb_in", [T_STEPS, 128, NGD], f32, kind="ExternalInput")
    p_loc_in = nc.dram_tensor("p_loc_in", [T_STEPS, 1, J_LOC], f32, kind="ExternalInput")
    ones_col_in = nc.dram_tensor("ones_col_in", [128, 1], f16, kind="ExternalInput")
    ones_row_in = nc.dram_tensor("ones_row_in", [1, 128], f16, kind="ExternalInput")
    ident_in = nc.dram_tensor("ident_in", [32, 32], f32, kind="ExternalInput")
    loss_out = nc.dram_tensor("loss_out", [1, 1], f32, kind="ExternalOutput")
    if debug_h:
        hdbg_out = nc.dram_tensor("hdbg_out", [t_run * n_iter, 1, J_LOC], f32,
                                  kind="ExternalOutput")

    with tile.TileContext(nc) as tc:
        with (
            tc.tile_pool(name="state_pool", bufs=1) as state_pool,
            tc.tile_pool(name="h_pool", bufs=3) as h_pool,
            tc.tile_pool(name="hr_pool", bufs=2) as hr_pool,
            tc.tile_pool(name="acc_pool", bufs=2) as acc_pool,
            tc.tile_pool(name="pw_pool", bufs=3) as pw_pool,
            tc.tile_pool(name="hT_pool", bufs=2) as hT_pool,
            tc.tile_pool(name="psum_pool", bufs=2, space="PSUM") as psum_pool,
            tc.tile_pool(name="tr_psum_pool", bufs=2, space="PSUM") as tr_psum_pool,
            tc.tile_pool(name="vb_psum_pool", bufs=2, space="PSUM") as vb_psum_pool,
            tc.tile_pool(name="dram_pool", bufs=1, space="DRAM") as dram_pool,
        ):
            # ---- persistent SBUF state ----
            m16 = state_pool.tile([128, NCHUNK * J_LOC], f16)
            v_bcast16 = state_pool.tile([128, J_LOC], f16)
            u_eta = state_pool.tile([128, NGD], f32)
            p_sb = state_pool.tile([128, NGD], f32)
            ones_col16 = state_pool.tile([128, 1], f16)
            ones_row16 = state_pool.tile([1, 128], f16)
            ident32 = state_pool.tile([32, 32], f32)
            loss_acc = state_pool.tile([1, 1], f32)
            loss_tmp = state_pool.tile([1, 1], f32)

            cc_in = [dram_pool.tile([1, 512], f32, name=f"cc_in{h}", tag=f"cc_in{h}")
                     for h in range(2)]

            # ---- init ----
            nc.gpsimd.memset(loss_acc[:], 0.0)
            nc.sync.dma_start(ones_col16[:], ones_col_in[:])
            nc.sync.dma_start(ones_row16[:], ones_row_in[:])
            nc.sync.dma_start(ident32[:], ident_in[:])
            n_ld = 8
            step = (NCHUNK * J_LOC) // n_ld
            for i in range(n_ld):
                nc.sync.dma_start(m16[:, i * step:(i + 1) * step],
                                  m16_in[:, i * step:(i + 1) * step])

            def make_asm(cc_out, h32_next, h16_next, half, t, it):
                """Deferred assembly: DMA gathered h -> transpose -> copies."""
                def emit():
                    hT = hT_pool.tile([32, 128], f32, tag="hT",
                                      name=f"hT_{t}_{it}_{half}")
                    cc_v = cc_out[:].rearrange("o (q p) -> (o q) p", p=128)
                    nc.sync.dma_start(hT[:], cc_v)
                    tr = tr_psum_pool.tile([128, 32], f32, tag="tr",
                                           name=f"tr_{t}_{it}_{half}")
                    nc.tensor.transpose(tr[:], hT[:], ident32[:])
                    cols = slice(half * 32, (half + 1) * 32)
                    nc.scalar.activation(h32_next[:, cols], tr[:], AF.Lrelu,
                                         alpha=float(NEG))
                    nc.scalar.activation(h16_next[:, cols], tr[:], AF.Lrelu,
                                         alpha=float(NEG))
                return emit

            for t in range(t_run):
                scale_t = float(LAMDA ** t)

                h32 = h_pool.tile([128, NGD], f32, tag="h32", name=f"h32_{t}_0")
                nc.sync.dma_start(h32[:], h0_sb_in[t])
                h16 = h_pool.tile([128, NGD], f16, tag="h16", name=f"h16_{t}_0")
                nc.vector.tensor_copy(h16[:], h32[:])
                h_row = hr_pool.tile([1, J_LOC], f32, tag="hr", name=f"hr_{t}_0")
                nc.sync.dma_start(h_row[:], h0_row_in[t])

                pending = [None, None]

                for it in range(n_iter):
                    with nc.named_scope(f"iter_t{t}_i{it}"):
                        acc_ps = psum_pool.tile([1, J_LOC], f32, tag="acc",
                                                name=f"acc_{t}_{it}")
                        acc16 = acc_pool.tile([128, J_LOC], f16, tag="acc16",
                                              name=f"acc16_{t}_{it}")
                        h32_next = h_pool.tile([128, NGD], f32, tag="h32",
                                               name=f"h32_{t}_{it + 1}")
                        h16_next = h_pool.tile([128, NGD], f16, tag="h16",
                                               name=f"h16_{t}_{it + 1}")
                        h_new = hr_pool.tile([1, J_LOC], f32, tag="hr",
                                             name=f"hr_{t}_{it + 1}")

                        def dve_block(chunks, half, init):
                            cs = slice(half * 512, (half + 1) * 512)
                            for n, c in enumerate(chunks):
                                mc = m16[:, c * J_LOC + half * 512:
                                         c * J_LOC + half * 512 + 512]
                                hcol = h32[:, c:c + 1]
                                if init and n == 0:
                                    nc.vector.tensor_scalar_mul(acc16[:, cs], mc, hcol)
                                else:
                                    nc.vector.scalar_tensor_tensor(
                                        acc16[:, cs], mc, hcol, acc16[:, cs],
                                        ALU.mult, ALU.add)

                        def pe_block(chunks, half, start):
                            cs = slice(half * 512, (half + 1) * 512)
                            for n, c in enumerate(chunks):
                                mc = m16[:, c * J_LOC + half * 512:
                                         c * J_LOC + half * 512 + 512]
                                nc.tensor.matmul(acc_ps[:, cs], h16[:, c:c + 1], mc,
                                                 start=(start and n == 0), stop=False,
                                                 skip_group_check=True)

                        def finish_half(half):
                            cs = slice(half * 512, (half + 1) * 512)
                            # partition-reduce the DVE accumulator into the bank
                            nc.tensor.matmul(acc_ps[:, cs], ones_col16[:],
                                             acc16[:, cs], start=False, stop=True,
                                             skip_group_check=True)
                            # pointwise: h = f_p(h*(lamda^t*raw + kappa))
                            s_t = pw_pool.tile([1, 512], f32, tag="pw",
                                               name=f"s_{t}_{it}_{half}")
                            nc.scalar.activation(s_t[:], acc_ps[:, cs], AF.Identity,
                                                 bias=float(KAPPA), scale=scale_t)
                            w = pw_pool.tile([1, 512], f32, tag="pw",
                                             name=f"w_{t}_{it}_{half}")
                            nc.vector.tensor_tensor(w[:], h_row[:, cs], s_t[:],
                                                    ALU.mult)
                            wc = pw_pool.tile([1, 512], f32, tag="pw",
                                              name=f"wc_{t}_{it}_{half}")
                            nc.vector.tensor_scalar(wc[:], w[:], 1.0, -1.0,
                                                    ALU.min, ALU.max)
                            # exchange the clipped pre-lrelu values; lrelu is
                            # applied by the asm copies and locally below
                            nc.sync.dma_start(cc_in[half][:], wc[:])
                            nc.scalar.activation(h_new[:, cs], wc[:], AF.Lrelu,
                                                 alpha=float(NEG))
                            cc_out = dram_pool.tile([1, 4096], f32,
                                                    addr_space="Shared",
                                                    name=f"cc_out_{t}_{it}_{half}",
                                                    tag=f"cc_out_{t}_{it}_{half}")
                            nc.gpsimd.collective_compute(
                                "AllGather", ALU.bypass,
                                replica_groups=[list(range(N_CORES))],
                                ins=[cc_in[half][:].opt()],
                                outs=[cc_out[:].opt()],
                            )
                            return cc_out

                        # --- software-pipelined emission ---
                        if pending[0] is not None:
                            pending[0]()
                        dve_block(DVE_G0, 0, init=True)
                        pe_block(PE_G0, 0, start=True)
                        if pending[1] is not None:
                            pending[1]()
                        dve_block(DVE_G1, 0, init=False)
                        pe_block(PE_G1, 0, start=False)
                        cc0 = finish_half(0)
                        dve_block(DVE_G0, 1, init=True)
                        pe_block(PE_G0, 1, start=True)
                        dve_block(DVE_G1, 1, init=False)
                        pe_block(PE_G1, 1, start=False)
                        cc1 = finish_half(1)
                        if debug_h:
                            nc.sync.dma_start(hdbg_out[t * n_iter + it], h_new[:])

                        pending = [make_asm(cc0, h32_next, h16_next, 0, t, it),
                                   make_asm(cc1, h32_next, h16_next, 1, t, it)]
                        h32 = h32_next
                        h16 = h16_next
                        h_row = h_new

                # ---- timestep tail: final assembly, loss, Hebbian update ----
                pending[0]()
                pending[1]()
                p_loc = pw_pool.tile([1, J_LOC], f32, tag="pw", name=f"ploc_{t}")
                nc.sync.dma_start(p_loc[:], p_loc_in[t])
                v_row = pw_pool.tile([1, J_LOC], f32, tag="pw", name=f"vrow_{t}")
                nc.vector.tensor_tensor(v_row[:], p_loc[:], h_row[:], ALU.subtract)
                nc.vector.tensor_reduce(loss_tmp[:], v_row[:],
                                        mybir.AxisListType.X, ALU.add,
                                        apply_absolute_value=True)
                nc.vector.tensor_tensor(loss_acc[:], loss_acc[:], loss_tmp[:], ALU.add)

                if t < t_run - 1:
                    coef = float(YITA / (LAMDA ** (t + 1)))
                    nc.sync.dma_start(p_sb[:], p_sb_in[t])
                    nc.vector.tensor_tensor(u_eta[:], p_sb[:], h32[:], ALU.add)
                    nc.vector.tensor_scalar_mul(u_eta[:], u_eta[:], coef)
                    v16_row = pw_pool.tile([1, J_LOC], f16, tag="pw16",
                                           name=f"v16_{t}")
                    nc.vector.tensor_copy(v16_row[:], v_row[:])
                    for half in range(2):
                        cs = slice(half * 512, (half + 1) * 512)
                        vb_ps = vb_psum_pool.tile([128, 512], f32, tag="vb",
                                                  name=f"vb_{t}_{half}")
                        nc.tensor.matmul(vb_ps[:], ones_row16[:], v16_row[:, cs],
                                         start=True, stop=True)
                        nc.vector.tensor_copy(v_bcast16[:, cs], vb_ps[:])
                    for c in range(NCHUNK):
                        mc = m16[:, c * J_LOC:(c + 1) * J_LOC]
                        nc.vector.scalar_tensor_tensor(mc, v_bcast16[:],
                                                       u_eta[:, c:c + 1], mc,
                                                       ALU.mult, ALU.add)

            nc.sync.dma_start(loss_out[:], loss_acc[:])

    nc.compile()
    return nc


def prepare_inputs(x, g, M0):
    """Host-side sharding/layout prep. Returns list of per-core input maps."""
    x = np.asarray(x, dtype=np.float32)
    g = np.asarray(g, dtype=np.float32)
    M0 = np.ascontiguousarray(np.asarray(M0, dtype=np.float32))

    perm = np.array([g_perm(c) for c in range(NCHUNK)])
    Mv = M0.reshape(NCHUNK, 128, P_DIM)[perm]          # [c, p, col]

    h0_sb = np.zeros((T_STEPS, 128, NGD), np.float32)
    p_sb = np.zeros((T_STEPS, 128, NGD), np.float32)
    h0_flat = np.zeros((T_STEPS, P_DIM), np.float32)
    p_flat = np.zeros((T_STEPS, P_DIM), np.float32)
    for t in range(T_STEPS):
        h0 = _f_p(np.tile(g[t], NXD))
        p = np.outer(x[t], g[t]).reshape(P_DIM).astype(np.float32)
        h0_flat[t] = h0
        p_flat[t] = p
        h0_sb[t] = h0.reshape(NCHUNK, 128)[perm].T
        p_sb[t] = p.reshape(NCHUNK, 128)[perm].T

    ones_col = np.ones((128, 1), np.float16)
    ones_row = np.ones((1, 128), np.float16)
    ident = np.eye(32, dtype=np.float32)

    in_maps = []
    for k in range(N_CORES):
        shard = Mv[:, :, k * J_LOC:(k + 1) * J_LOC]    # [64, 128, 1024]
        m16 = np.ascontiguousarray(shard.transpose(1, 0, 2)).reshape(
            128, NCHUNK * J_LOC).astype(np.float16)
        in_maps.append({
            "m16_in": m16,
            "h0_sb_in": h0_sb,
            "h0_row_in": h0_flat[:, k * J_LOC:(k + 1) * J_LOC].reshape(
                T_STEPS, 1, J_LOC).copy(),
            "p_sb_in": p_sb,
            "p_loc_in": p_flat[:, k * J_LOC:(k + 1) * J_LOC].reshape(
                T_STEPS, 1, J_LOC).copy(),
            "ones_col_in": ones_col,
            "ones_row_in": ones_row,
            "ident_in": ident,
        })
    return in_maps


def kernel(x, g, M0):
    from concourse.bass_utils import run_bass_kernel_spmd

    in_maps = prepare_inputs(x, g, M0)
    if "nc" not in _cache:
        _cache["nc"] = build_program()
    nc = _cache["nc"]
    trace = bool(int(os.environ.get("MG_TRACE", "0")))
    res = run_bass_kernel_spmd(nc, in_maps, core_ids=list(range(N_CORES)),
                               trace=trace)
    _cache["last_result"] = res
    total = np.float32(0.0)
    for k in range(N_CORES):
        total += res.results[k]["loss_out"][0, 0]
    return np.float32(total)
